# revision 27
# baseline (speedup 1.0000x reference)
"""CharLevelEncoder Trainium2 kernel (8-core SPMD).

Math: out = relu(concat(word_emb[word_ids], h(char_ids)) @ W_lin.T + b_lin)
with h a single LSTM cell step from zero state on E[char_ids].

Algebraic restructuring:
  - h depends only on char_id (40 values) -> HB table [40, WD]:
        HB[c] = h_c @ W_lin[:, WD:].T + b_lin
  - word_emb[word_ids] @ W_lin[:, :WD].T == (word_emb @ A.T)[word_ids], A = W_lin[:, :WD]
  - out[t] = relu(P[word_ids[t]] + HB[char_ids[t]])

Device mapping (PE matmul cost on trn2 is out_free_size cycles regardless of
contraction K, so matmul COUNT is the currency):
  - words are packed into sub-bins of <=64 words and <=256 chars; a chunk is a
    pair of sub-bins (L at slots 0-63, R at slots 64-127).  P_chunk [128, WD]
    is computed with 16 matmuls (full 128-slot efficiency).
  - the P PSUM halves drain into two combined tables:
        CL = [P_L (rows 0-63); HB (rows 64-103); 0]
        CR = [HB (rows 0-39); 0; P_R (rows 64-127)]
    (HB rows written once per persistent buffer; P rows rewritten per chunk).
  - each 128-char tile needs ONE matmul per 512-wide half: the stationary
    one-hot G column for char q has TWO ones - at its word-slot row and at its
    char-id row - so P-gather and HB-add happen in a single K=128 pass.
    Expansion cost: 2 matmuls/tile (vs 4 with separate sel/oc matmuls).
  - relu on ACT/DVE, output stored bf16 (halves store DMA), host casts to f32
    and inverse-permutes.
"""

import ml_dtypes
import numpy as np

import concourse.bass as bass
import concourse.tile as tile
from concourse import bacc, mybir
from concourse.bass_utils import run_bass_kernel_spmd

NCORES = 8
WD = 1024
NE = 40
SUB_WORD_CAP = 64
SUB_CHAR_CAP = 256

MM_DT = mybir.dt.bfloat16
NP_DT = ml_dtypes.bfloat16


def _sigmoid(x):
    return 1.0 / (1.0 + np.exp(-x))


def _hb_table(E, W_ih, b_ih, b_hh, W_lin, b_lin):
    G = E.astype(np.float32) @ W_ih.T + b_ih + b_hh  # [NE, 4H]
    i, f, g, o = np.split(G, 4, axis=1)
    c = _sigmoid(i) * np.tanh(g)
    h = _sigmoid(o) * np.tanh(c)  # [NE, H]
    return (h @ W_lin[:, WD:].T + b_lin).astype(np.float32)  # [NE, WD]


def _pack_bins(word_ids, n_words):
    """Pack words into sub-bins of <=SUB_WORD_CAP words / <=SUB_CHAR_CAP chars.
    Snake-deal of words sorted by char count keeps bin char totals equal."""
    wc = np.bincount(word_ids, minlength=n_words)
    used = np.nonzero(wc)[0]
    counts = wc[used]
    assert counts.max() <= SUB_CHAR_CAP, "single word exceeds bin capacity"
    order = np.argsort(-counts, kind="stable")
    wsorted = used[order]
    csorted = counts[order]
    nused = len(used)
    total = int(counts.sum())

    group = 2 * NCORES  # sub-bins per (chunk row across all cores)
    nbins = max(int(np.ceil(nused / (SUB_WORD_CAP - 2))),
                int(np.ceil(total / (SUB_CHAR_CAP - 4))))
    nbins = ((nbins + group - 1) // group) * group
    while True:
        bin_of = np.empty(nused, np.int32)
        for r in range(int(np.ceil(nused / nbins))):
            lo, hi = r * nbins, min((r + 1) * nbins, nused)
            idx = np.arange(lo, hi)
            if r % 2 == 0:
                bin_of[idx] = idx - lo
            else:
                bin_of[idx] = nbins - 1 - (idx - lo)
        sums = np.bincount(bin_of, weights=csorted, minlength=nbins)
        nword = np.bincount(bin_of, minlength=nbins)
        if sums.max() <= SUB_CHAR_CAP and nword.max() <= SUB_WORD_CAP:
            break
        nbins += group
    # Repair pass: concentrate slack into the smallest bins so most sub-bins
    # fill whole 128-char tiles (fewer tiles => less padding).
    import bisect
    ndon = min(4 * group, nbins)
    order_bins = np.argsort(-sums, kind="stable")
    donors = set(order_bins[nbins - ndon:].tolist())
    pool = sorted((int(csorted[j]), j) for j in range(nused)
                  if int(bin_of[j]) in donors)  # asc by count
    keys = [c for c, _ in pool]
    for b in order_bins[:nbins - ndon]:
        deficit = int(SUB_CHAR_CAP - sums[b])
        while deficit > 0 and nword[b] < SUB_WORD_CAP and pool:
            k = bisect.bisect_right(keys, deficit) - 1
            if k < 0:
                break
            c, j = pool.pop(k)
            keys.pop(k)
            src = int(bin_of[j])
            bin_of[j] = b
            sums[b] += c
            sums[src] -= c
            nword[b] += 1
            nword[src] -= 1
            deficit -= c

    word_bin = np.full(n_words, -1, np.int32)
    word_bin[wsorted] = bin_of
    word_slot = np.full(n_words, -1, np.int32)
    ord2 = np.argsort(bin_of, kind="stable")
    slots = np.arange(nused) - np.concatenate(
        [[0], np.cumsum(np.bincount(bin_of, minlength=nbins))])[bin_of[ord2]]
    word_slot[wsorted[ord2]] = slots
    return word_bin, word_slot, nbins, sums, nword


def _build_program(nch, tiles_L, tiles_R, tile_base):
    ntiles = int(tile_base[-1])
    f32 = mybir.dt.float32
    nc = bacc.Bacc("TRN2", target_bir_lowering=False, debug=False,
                   num_devices=NCORES)
    weTb_ap = nc.dram_tensor("weTb", [nch, 128, WD], MM_DT,
                             kind="ExternalInput").ap()
    at_ap = nc.dram_tensor("atb", [128, 8 * WD], MM_DT,
                           kind="ExternalInput").ap()
    G_ap = nc.dram_tensor("gmat", [128, ntiles * 128], MM_DT,
                          kind="ExternalInput").ap()
    HB_ap = nc.dram_tensor("hbt", [NE, WD], MM_DT, kind="ExternalInput").ap()
    out_ap = nc.dram_tensor("out", [ntiles * 128, WD], MM_DT,
                            kind="ExternalOutput").ap()

    # chunk ranges for resident weight/one-hot pieces (finer up front so the
    # PE can start early)
    pieces = []
    c0 = 0
    for sz in (1, 1, 2, 4):
        if c0 >= nch:
            break
        sz = min(sz, nch - c0)
        pieces.append((c0, c0 + sz))
        c0 += sz
    while c0 < nch:
        sz = min(4, nch - c0)
        pieces.append((c0, c0 + sz))
        c0 += sz
    piece_of = np.empty(nch, np.int64)
    for pi, (a, b) in enumerate(pieces):
        piece_of[a:b] = pi

    with tile.TileContext(nc) as tc:
        with tc.tile_pool(name="at", bufs=1) as atp, \
             tc.tile_pool(name="wb", bufs=1) as wbp, \
             tc.tile_pool(name="gm", bufs=1) as gmp, \
             tc.tile_pool(name="cl", bufs=1) as clp, \
             tc.tile_pool(name="cr", bufs=1) as crp, \
             tc.tile_pool(name="ob", bufs=4) as obp, \
             tc.tile_pool(name="ps_pre", bufs=2, space="PSUM") as pspre, \
             tc.tile_pool(name="ps_exp", bufs=2, space="PSUM") as psexp:
            # A.T as 4 independent tiles so the first P matmuls only wait on
            # the slice they read: ats[n*2 + kh] covers column-half n,
            # k-slices 4*kh..4*kh+3
            ats = [atp.tile([128, 2 * WD], MM_DT, tag=f"at{m}", name=f"at{m}")
                   for m in range(4)]

            def at_slice(k, n):
                t = ats[n * 2 + k // 4]
                return t[:, (k % 4) * 512:(k % 4) * 512 + 512]

            # combined tables: memset + one HB DMA, then on-chip replication
            # (DVE is idle this early; keeps the scalar DMA queue short)
            cls = [clp.tile([128, WD], MM_DT, tag=f"cl{i}", name=f"cl{i}")
                   for i in range(3)]
            crs = [crp.tile([128, WD], MM_DT, tag=f"cr{i}", name=f"cr{i}")
                   for i in range(3)]
            # split the input feed across both HWDGE rings, each in its
            # consumption order: sync carries weights + A.T, scalar carries
            # the first A.T piece, HB, and the one-hot pieces.  The scalar
            # engine issues all its dma_starts up-front, before any drain
            # instructions reach its queue.
            nc.scalar.dma_start(ats[0][:], at_ap[:, 0:2 * WD])
            for i in range(3):
                nc.vector.memset(cls[i][64:128, :], 0.0)
                nc.vector.memset(crs[i][0:64, :], 0.0)
            nc.scalar.dma_start(cls[0][64:64 + NE, :], HB_ap[:])
            nc.scalar.dma_start(crs[0][0:NE, :], HB_ap[:])
            for i in (1, 2):
                nc.vector.tensor_copy(cls[i][64:128, :], cls[0][64:128, :])
                nc.vector.tensor_copy(crs[i][0:64, :], crs[0][0:64, :])

            wbs = []
            gms = []
            for pi, (a, b) in enumerate(pieces):
                wb_t = wbp.tile([128, (b - a) * WD], MM_DT, tag=f"wb{pi}",
                                name=f"wb{pi}")
                wbs.append(wb_t)
                nc.sync.dma_start(
                    wb_t[:].rearrange("p (c f) -> p c f", f=WD),
                    weTb_ap[a:b].rearrange("c p f -> p c f"))
                ga, gb = int(tile_base[a]) * 128, int(tile_base[b]) * 128
                gm_t = None
                if gb > ga:
                    gm_t = gmp.tile([128, gb - ga], MM_DT, tag=f"gm{pi}",
                                    name=f"gm{pi}")
                    nc.scalar.dma_start(gm_t[:], G_ap[:, ga:gb])
                gms.append(gm_t)
                if pi == 0:
                    nc.sync.dma_start(ats[2][:], at_ap[:, 4 * WD:6 * WD])
                elif pi == 1:
                    nc.sync.dma_start(ats[1][:], at_ap[:, 2 * WD:4 * WD])
                    nc.sync.dma_start(ats[3][:], at_ap[:, 6 * WD:8 * WD])

            def make_P(c, first=False):
                """Return (16 matmul closures, drain closure) for chunk c."""
                pi = int(piece_of[c])
                a, _ = pieces[pi]
                wb = wbs[pi]
                wcol = (c - a) * WD
                cl = cls[c % 3]
                cr = crs[c % 3]
                # both 512-col halves accumulate in one [128,1024] PSUM tile
                # (2 banks); k-interleaved so consecutive matmuls share a
                # stationary operand
                pp = pspre.tile([128, WD], f32, space="PSUM", name="pp",
                                tag="pp")

                def mm(k, n):
                    def go():
                        nc.tensor.matmul(
                            pp[:, n * 512:(n + 1) * 512],
                            wb[:, wcol + k * 128: wcol + (k + 1) * 128],
                            at_slice(k, n),
                            start=(k == 0), stop=(k == 7))
                    return go

                def drain():
                    # full-width drains amortize the fixed PSUM-access cost;
                    # GPSIMD cannot read PSUM so split across ACT and DVE
                    nc.scalar.copy(cl[0:64, :], pp[0:64, :])
                    nc.vector.tensor_copy(cr[64:128, :], pp[64:128, :])

                if first:
                    # match the order the A.T pieces arrive from DRAM
                    order = ([(k, 0) for k in range(4)]
                             + [(k, 1) for k in range(4)]
                             + [(k, 0) for k in range(4, 8)]
                             + [(k, 1) for k in range(4, 8)])
                else:
                    order = [(k, n) for k in range(8) for n in range(2)]
                return [mm(k, n) for k, n in order], drain

            tg = 0
            chunks = [c for c in range(nch)
                      if int(tiles_L[c]) + int(tiles_R[c]) > 0]

            def make_exp(c):
                """Expansion tile closures + store closure for chunk c."""
                nonlocal tg
                tL, tR = int(tiles_L[c]), int(tiles_R[c])
                tpc = tL + tR
                pi = int(piece_of[c])
                a, _ = pieces[pi]
                gm = gms[pi]
                gcol0 = int(tile_base[a]) * 128
                cl = cls[c % 3]
                cr = crs[c % 3]
                ob = obp.tile([128, tpc * WD], MM_DT, tag="ob", name="ob")
                tgc = tg
                tiles = []
                ti = 0
                for src, cnt in ((cl, tL), (cr, tR)):
                    for _ in range(cnt):
                        def tile_go(src=src, ti=ti, gc=tg * 128 - gcol0,
                                    last=False):
                            pe = psexp.tile([128, WD], f32, space="PSUM",
                                            name="pe")
                            for n in range(2):
                                nc.tensor.matmul(
                                    pe[:, n * 512:(n + 1) * 512],
                                    gm[:, gc:gc + 128],
                                    src[:, n * 512:(n + 1) * 512],
                                    start=True, stop=True)
                            dst = ob[:, ti * WD: (ti + 1) * WD]
                            if not last:
                                if ti % 2 == 0:
                                    nc.scalar.activation(
                                        dst, pe[:],
                                        mybir.ActivationFunctionType.Relu)
                                else:
                                    nc.vector.tensor_scalar_max(dst, pe[:],
                                                                0.0)
                            else:
                                # tail: drain the halves on both engines in
                                # parallel and store via the idle sync ring
                                nc.scalar.activation(
                                    dst[:, 0:512], pe[:, 0:512],
                                    mybir.ActivationFunctionType.Relu)
                                nc.vector.tensor_scalar_max(
                                    dst[:, 512:1024], pe[:, 512:1024], 0.0)
                                t0 = (tgc + ti) * 128
                                nc.sync.dma_start(out_ap[t0:t0 + 128, :],
                                                  dst)
                        tiles.append(tile_go)
                        ti += 1
                        tg += 1

                def store(last):
                    if not last:
                        dram = out_ap[tgc * 128: (tgc + tpc) * 128, :].rearrange(
                            "(i p) f -> p i f", p=128)
                        nc.gpsimd.dma_start(
                            dram, ob[:].rearrange("p (i f) -> p i f", f=WD))

                return tiles, store

            # software pipeline: chunk c's expansion tiles are interleaved
            # into chunk c+1's P chain (~3 P matmuls per exp tile), so the PE
            # stream never waits on PSUM drains.
            mms, drain = make_P(chunks[0], first=True)
            for go in mms:
                go()
            drain()
            for ci, c in enumerate(chunks):
                tiles, store = make_exp(c)
                if ci + 1 < len(chunks):
                    mms, drain = make_P(chunks[ci + 1])
                    lead = 4
                    for go in mms[:lead]:
                        go()
                    rem = mms[lead:]
                    nt = len(tiles)
                    done = 0
                    for j, tgo in enumerate(tiles):
                        upto = ((j + 1) * len(rem)) // (nt + 1)
                        while done < upto:
                            rem[done]()
                            done += 1
                        tgo()
                    while done < len(rem):
                        rem[done]()
                        done += 1
                    drain()
                else:
                    for tgo in tiles:
                        tgo(last=True)
                store(ci == len(chunks) - 1)
    nc.compile()
    return nc


def kernel(word_emb, char_ids, word_ids, E, W_ih, b_ih, b_hh, W_lin, b_lin,
           _timing=None, _trace_cores=None):
    word_emb = np.asarray(word_emb, np.float32)
    char_ids = np.asarray(char_ids, np.int32)
    word_ids = np.asarray(word_ids, np.int32)
    E = np.asarray(E, np.float32)
    W_ih = np.asarray(W_ih, np.float32)
    b_ih = np.asarray(b_ih, np.float32)
    b_hh = np.asarray(b_hh, np.float32)
    W_lin = np.asarray(W_lin, np.float32)
    b_lin = np.asarray(b_lin, np.float32)

    T = char_ids.shape[0]
    NW = word_emb.shape[0]

    HB = _hb_table(E, W_ih, b_ih, b_hh, W_lin, b_lin)
    A = np.ascontiguousarray(W_lin[:, :WD])

    word_bin, word_slot, nbins, bin_chars, bin_words = _pack_bins(word_ids, NW)
    assert bin_words.max() <= SUB_WORD_CAP and bin_chars.max() <= SUB_CHAR_CAP

    # deal sub-bins to (core, slot) by descending char count; slot ->
    # (chunk, half).  Rank ordering keeps per-slot char counts uniform across
    # cores so the shared tiles-per-slot wastes little padding.
    rank_of_bin = np.empty(nbins, np.int64)
    rank_of_bin[np.argsort(-bin_chars, kind="stable")] = np.arange(nbins)
    core_of_bin = (rank_of_bin % NCORES).astype(np.int32)
    slot_of_bin = (rank_of_bin // NCORES).astype(np.int32)
    nslots = nbins // NCORES
    assert nslots % 2 == 0
    nch = nslots // 2

    # chars sorted by (bin, word slot) -> contiguous per bin, word-major
    cb = word_bin[word_ids]
    cslot = word_slot[word_ids]
    ckey = cb.astype(np.int64) * 512 + cslot
    corder = np.argsort(ckey, kind="stable")
    per_bin = np.bincount(cb, minlength=nbins)
    bstart = np.concatenate([[0], np.cumsum(per_bin)])

    slot_cnt = np.zeros((NCORES, nslots), np.int64)
    for b in range(nbins):
        slot_cnt[core_of_bin[b], slot_of_bin[b]] = per_bin[b]
    tiles_per_slot = np.ceil(slot_cnt.max(axis=0) / 128).astype(np.int64)
    # drop empty trailing chunks
    while nch > 1 and tiles_per_slot[2 * nch - 2: 2 * nch].sum() == 0:
        nch -= 1
    nslots = 2 * nch
    tiles_per_slot = tiles_per_slot[:nslots]
    tiles_L = tiles_per_slot[0::2]
    tiles_R = tiles_per_slot[1::2]
    ntiles = int(tiles_per_slot.sum())
    slot_tile_base = np.concatenate([[0], np.cumsum(tiles_per_slot)])
    # chunk tile base for the device program piece layout
    chunk_tiles = tiles_L + tiles_R
    chunk_tile_base = np.concatenate([[0], np.cumsum(chunk_tiles)])

    AT3 = np.ascontiguousarray(A.T.reshape(8, 128, WD))
    at_host = np.empty((128, 8 * WD), NP_DT)
    for n in range(2):
        for k in range(8):
            at_host[:, n * 4 * WD + k * 512: n * 4 * WD + (k + 1) * 512] = \
                AT3[k][:, n * 512:(n + 1) * 512].astype(NP_DT)
    HBq = HB.astype(NP_DT)

    bin_of_cs = {}  # (core, slot) -> bin id
    for b in range(nbins):
        bin_of_cs[(int(core_of_bin[b]), int(slot_of_bin[b]))] = b

    in_maps = []
    origs = []
    for m in range(NCORES):
        weTb = np.zeros((nch, 128, WD), NP_DT)
        G = np.zeros((128, ntiles * 128), NP_DT)
        orig = np.full(ntiles * 128, -1, np.int64)
        for s in range(nslots):
            b = bin_of_cs.get((m, s))
            if b is None:
                continue
            c, half = s // 2, s % 2
            lo, hi = bstart[b], bstart[b + 1]
            chars = corder[lo:hi]
            wlist = np.nonzero(word_bin == b)[0]
            wlist = wlist[np.argsort(word_slot[wlist])]
            nwb = len(wlist)
            if nwb:
                rows = word_emb[wlist]  # [nwb, WD]
                blk = rows.T.reshape(8, 128, nwb).transpose(1, 0, 2)
                weTb[c].reshape(128, 8, 128)[
                    :, :, 64 * half: 64 * half + nwb] = blk.astype(NP_DT)
            q = np.arange(len(chars))
            col = slot_tile_base[s] * 128 + q
            ws = cslot[chars]
            cid = char_ids[chars]
            if half == 0:   # L sub-bin: sel rows 0-63, HB rows 64-103
                G[ws, col] = 1.0
                G[64 + cid, col] = 1.0
            else:           # R sub-bin: sel rows 64-127, HB rows 0-39
                G[64 + ws, col] = 1.0
                G[cid, col] = 1.0
            orig[col] = chars
        in_maps.append({
            "weTb": weTb,
            "atb": at_host,
            "gmat": G,
            "hbt": HBq,
        })
        origs.append(orig)

    nc = _build_program(nch, tiles_L, tiles_R, chunk_tile_base)
    kwargs = {}
    if _trace_cores is not None:
        kwargs = dict(trace=True, trace_cores=_trace_cores)
    res = run_bass_kernel_spmd(nc, in_maps, core_ids=list(range(NCORES)),
                               **kwargs)
    if _timing is not None:
        _timing["exec_time_ns"] = res.exec_time_ns
        _timing["results"] = res

    out = np.empty((T, WD), np.float32)
    for m in range(NCORES):
        o = np.asarray(res.results[m]["out"]).astype(np.float32)
        v = origs[m] >= 0
        out[origs[m][v]] = o[v]
    return out


# revision 28
# speedup vs baseline: 1.0671x; 1.0671x over previous
"""CharLevelEncoder Trainium2 kernel (8-core SPMD).

Math: out = relu(concat(word_emb[word_ids], h(char_ids)) @ W_lin.T + b_lin)
with h a single LSTM cell step from zero state on E[char_ids].

Algebraic restructuring:
  - h depends only on char_id (40 values) -> HB table [40, WD]:
        HB[c] = h_c @ W_lin[:, WD:].T + b_lin
  - word_emb[word_ids] @ W_lin[:, :WD].T == (word_emb @ A.T)[word_ids], A = W_lin[:, :WD]
  - out[t] = relu(P[word_ids[t]] + HB[char_ids[t]])

Device mapping (PE matmul cost on trn2 is out_free_size cycles regardless of
contraction K, so matmul COUNT is the currency):
  - words are packed into sub-bins of <=64 words and <=256 chars; a chunk is a
    pair of sub-bins (L at slots 0-63, R at slots 64-127).  P_chunk [128, WD]
    is computed with 16 matmuls (full 128-slot efficiency).
  - the P PSUM halves drain into two combined tables:
        CL = [P_L (rows 0-63); HB (rows 64-103); 0]
        CR = [HB (rows 0-39); 0; P_R (rows 64-127)]
    (HB rows written once per persistent buffer; P rows rewritten per chunk).
  - each 128-char tile needs ONE matmul per 512-wide half: the stationary
    one-hot G column for char q has TWO ones - at its word-slot row and at its
    char-id row - so P-gather and HB-add happen in a single K=128 pass.
    Expansion cost: 2 matmuls/tile (vs 4 with separate sel/oc matmuls).
  - relu on ACT/DVE, output stored bf16 (halves store DMA), host casts to f32
    and inverse-permutes.
"""

import ml_dtypes
import numpy as np

import concourse.bass as bass
import concourse.tile as tile
from concourse import bacc, mybir
from concourse.bass_utils import run_bass_kernel_spmd

NCORES = 8
WD = 1024
NE = 40
SUB_WORD_CAP = 64
SUB_CHAR_CAP = 256

MM_DT = mybir.dt.bfloat16
NP_DT = ml_dtypes.bfloat16


def _sigmoid(x):
    return 1.0 / (1.0 + np.exp(-x))


def _hb_table(E, W_ih, b_ih, b_hh, W_lin, b_lin):
    G = E.astype(np.float32) @ W_ih.T + b_ih + b_hh  # [NE, 4H]
    i, f, g, o = np.split(G, 4, axis=1)
    c = _sigmoid(i) * np.tanh(g)
    h = _sigmoid(o) * np.tanh(c)  # [NE, H]
    return (h @ W_lin[:, WD:].T + b_lin).astype(np.float32)  # [NE, WD]


def _pack_bins(word_ids, n_words):
    """Pack words into sub-bins of <=SUB_WORD_CAP words / <=SUB_CHAR_CAP chars.
    Snake-deal of words sorted by char count keeps bin char totals equal."""
    wc = np.bincount(word_ids, minlength=n_words)
    used = np.nonzero(wc)[0]
    counts = wc[used]
    assert counts.max() <= SUB_CHAR_CAP, "single word exceeds bin capacity"
    order = np.argsort(-counts, kind="stable")
    wsorted = used[order]
    csorted = counts[order]
    nused = len(used)
    total = int(counts.sum())

    group = 2 * NCORES  # sub-bins per (chunk row across all cores)
    nbins = max(int(np.ceil(nused / (SUB_WORD_CAP - 2))),
                int(np.ceil(total / (SUB_CHAR_CAP - 4))))
    nbins = ((nbins + group - 1) // group) * group
    while True:
        bin_of = np.empty(nused, np.int32)
        for r in range(int(np.ceil(nused / nbins))):
            lo, hi = r * nbins, min((r + 1) * nbins, nused)
            idx = np.arange(lo, hi)
            if r % 2 == 0:
                bin_of[idx] = idx - lo
            else:
                bin_of[idx] = nbins - 1 - (idx - lo)
        sums = np.bincount(bin_of, weights=csorted, minlength=nbins)
        nword = np.bincount(bin_of, minlength=nbins)
        if sums.max() <= SUB_CHAR_CAP and nword.max() <= SUB_WORD_CAP:
            break
        nbins += group
    # Repair pass: concentrate slack into the smallest bins so most sub-bins
    # fill whole 128-char tiles (fewer tiles => less padding).
    import bisect
    ndon = min(4 * group, nbins)
    order_bins = np.argsort(-sums, kind="stable")
    donors = set(order_bins[nbins - ndon:].tolist())
    pool = sorted((int(csorted[j]), j) for j in range(nused)
                  if int(bin_of[j]) in donors)  # asc by count
    keys = [c for c, _ in pool]
    for b in order_bins[:nbins - ndon]:
        deficit = int(SUB_CHAR_CAP - sums[b])
        while deficit > 0 and nword[b] < SUB_WORD_CAP and pool:
            k = bisect.bisect_right(keys, deficit) - 1
            if k < 0:
                break
            c, j = pool.pop(k)
            keys.pop(k)
            src = int(bin_of[j])
            bin_of[j] = b
            sums[b] += c
            sums[src] -= c
            nword[b] += 1
            nword[src] -= 1
            deficit -= c

    word_bin = np.full(n_words, -1, np.int32)
    word_bin[wsorted] = bin_of
    word_slot = np.full(n_words, -1, np.int32)
    ord2 = np.argsort(bin_of, kind="stable")
    slots = np.arange(nused) - np.concatenate(
        [[0], np.cumsum(np.bincount(bin_of, minlength=nbins))])[bin_of[ord2]]
    word_slot[wsorted[ord2]] = slots
    return word_bin, word_slot, nbins, sums, nword


def _build_program(nch, tiles_L, tiles_R, tile_base):
    ntiles = int(tile_base[-1])
    f32 = mybir.dt.float32
    nc = bacc.Bacc("TRN2", target_bir_lowering=False, debug=False,
                   num_devices=NCORES)
    weTb_ap = nc.dram_tensor("weTb", [nch, 128, WD], MM_DT,
                             kind="ExternalInput").ap()
    at_ap = nc.dram_tensor("atb", [128, 8 * WD], MM_DT,
                           kind="ExternalInput").ap()
    G_ap = nc.dram_tensor("gmat", [128, ntiles * 128], MM_DT,
                          kind="ExternalInput").ap()
    HB_ap = nc.dram_tensor("hbt", [NE, WD], MM_DT, kind="ExternalInput").ap()
    out_ap = nc.dram_tensor("out", [ntiles * 128, WD], MM_DT,
                            kind="ExternalOutput").ap()

    # chunk ranges for resident weight/one-hot pieces (finer up front so the
    # PE can start early)
    pieces = []
    c0 = 0
    for sz in (1, 1, 2, 4):
        if c0 >= nch:
            break
        sz = min(sz, nch - c0)
        pieces.append((c0, c0 + sz))
        c0 += sz
    while c0 < nch:
        sz = min(4, nch - c0)
        pieces.append((c0, c0 + sz))
        c0 += sz
    piece_of = np.empty(nch, np.int64)
    for pi, (a, b) in enumerate(pieces):
        piece_of[a:b] = pi

    with tile.TileContext(nc) as tc:
        with tc.tile_pool(name="at", bufs=1) as atp, \
             tc.tile_pool(name="wb", bufs=1) as wbp, \
             tc.tile_pool(name="gm", bufs=1) as gmp, \
             tc.tile_pool(name="cl", bufs=1) as clp, \
             tc.tile_pool(name="cr", bufs=1) as crp, \
             tc.tile_pool(name="ob", bufs=4) as obp, \
             tc.tile_pool(name="ps_pre", bufs=2, space="PSUM") as pspre, \
             tc.tile_pool(name="ps_exp", bufs=2, space="PSUM") as psexp:
            # A.T as 4 independent tiles so the first P matmuls only wait on
            # the slice they read: ats[n*2 + kh] covers column-half n,
            # k-slices 4*kh..4*kh+3
            ats = [atp.tile([128, 2 * WD], MM_DT, tag=f"at{m}", name=f"at{m}")
                   for m in range(4)]

            def at_slice(k, n):
                t = ats[n * 2 + k // 4]
                return t[:, (k % 4) * 512:(k % 4) * 512 + 512]

            # combined tables: memset + one HB DMA, then on-chip replication
            # (DVE is idle this early; keeps the scalar DMA queue short)
            cls = [clp.tile([128, WD], MM_DT, tag=f"cl{i}", name=f"cl{i}")
                   for i in range(3)]
            crs = [crp.tile([128, WD], MM_DT, tag=f"cr{i}", name=f"cr{i}")
                   for i in range(3)]
            # split the input feed across both HWDGE rings, each in its
            # consumption order: sync carries weights + A.T, scalar carries
            # the first A.T piece, HB, and the one-hot pieces.  The scalar
            # engine issues all its dma_starts up-front, before any drain
            # instructions reach its queue.
            nc.scalar.dma_start(ats[0][:], at_ap[:, 0:2 * WD])
            for i in range(3):
                nc.vector.memset(cls[i][64:128, :], 0.0)
                nc.vector.memset(crs[i][0:64, :], 0.0)
            nc.scalar.dma_start(cls[0][64:64 + NE, :], HB_ap[:])
            nc.scalar.dma_start(crs[0][0:NE, :], HB_ap[:])
            for i in (1, 2):
                nc.vector.tensor_copy(cls[i][64:128, :], cls[0][64:128, :])
                nc.vector.tensor_copy(crs[i][0:64, :], crs[0][0:64, :])

            wbs = []
            gms = []
            for pi, (a, b) in enumerate(pieces):
                wb_t = wbp.tile([128, (b - a) * WD], MM_DT, tag=f"wb{pi}",
                                name=f"wb{pi}")
                wbs.append(wb_t)
                nc.sync.dma_start(
                    wb_t[:].rearrange("p (c f) -> p c f", f=WD),
                    weTb_ap[a:b].rearrange("c p f -> p c f"))
                ga, gb = int(tile_base[a]) * 128, int(tile_base[b]) * 128
                gm_t = None
                if gb > ga:
                    gm_t = gmp.tile([128, gb - ga], MM_DT, tag=f"gm{pi}",
                                    name=f"gm{pi}")
                    nc.sync.dma_start(gm_t[:], G_ap[:, ga:gb])
                gms.append(gm_t)
                if pi == 0:
                    nc.sync.dma_start(ats[2][:], at_ap[:, 4 * WD:6 * WD])
                elif pi == 1:
                    nc.sync.dma_start(ats[1][:], at_ap[:, 2 * WD:4 * WD])
                    nc.sync.dma_start(ats[3][:], at_ap[:, 6 * WD:8 * WD])

            def make_P(c, first=False):
                """Return (16 matmul closures, drain closure) for chunk c."""
                pi = int(piece_of[c])
                a, _ = pieces[pi]
                wb = wbs[pi]
                wcol = (c - a) * WD
                cl = cls[c % 3]
                cr = crs[c % 3]
                # both 512-col halves accumulate in one [128,1024] PSUM tile
                # (2 banks); k-interleaved so consecutive matmuls share a
                # stationary operand
                pp = pspre.tile([128, WD], f32, space="PSUM", name="pp",
                                tag="pp")

                def mm(k, n):
                    def go():
                        nc.tensor.matmul(
                            pp[:, n * 512:(n + 1) * 512],
                            wb[:, wcol + k * 128: wcol + (k + 1) * 128],
                            at_slice(k, n),
                            start=(k == 0), stop=(k == 7))
                    return go

                def drain():
                    # full-width drains amortize the fixed PSUM-access cost;
                    # GPSIMD cannot read PSUM so split across ACT and DVE
                    nc.scalar.copy(cl[0:64, :], pp[0:64, :])
                    nc.vector.tensor_copy(cr[64:128, :], pp[64:128, :])

                if first:
                    # match the order the A.T pieces arrive from DRAM
                    order = ([(k, 0) for k in range(4)]
                             + [(k, 1) for k in range(4)]
                             + [(k, 0) for k in range(4, 8)]
                             + [(k, 1) for k in range(4, 8)])
                else:
                    order = [(k, n) for k in range(8) for n in range(2)]
                return [mm(k, n) for k, n in order], drain

            tg = 0
            chunks = [c for c in range(nch)
                      if int(tiles_L[c]) + int(tiles_R[c]) > 0]

            def make_exp(c):
                """Expansion tile closures + store closure for chunk c."""
                nonlocal tg
                tL, tR = int(tiles_L[c]), int(tiles_R[c])
                tpc = tL + tR
                pi = int(piece_of[c])
                a, _ = pieces[pi]
                gm = gms[pi]
                gcol0 = int(tile_base[a]) * 128
                cl = cls[c % 3]
                cr = crs[c % 3]
                ob = obp.tile([128, tpc * WD], MM_DT, tag="ob", name="ob")
                tgc = tg
                tiles = []
                ti = 0
                for src, cnt in ((cl, tL), (cr, tR)):
                    for _ in range(cnt):
                        def tile_go(src=src, ti=ti, gc=tg * 128 - gcol0,
                                    last=False):
                            pe = psexp.tile([128, WD], f32, space="PSUM",
                                            name="pe")
                            for n in range(2):
                                nc.tensor.matmul(
                                    pe[:, n * 512:(n + 1) * 512],
                                    gm[:, gc:gc + 128],
                                    src[:, n * 512:(n + 1) * 512],
                                    start=True, stop=True)
                            dst = ob[:, ti * WD: (ti + 1) * WD]
                            if not last:
                                if ti % 2 == 0:
                                    nc.scalar.activation(
                                        dst, pe[:],
                                        mybir.ActivationFunctionType.Relu)
                                else:
                                    nc.vector.tensor_scalar_max(dst, pe[:],
                                                                0.0)
                            else:
                                # tail: drain the halves on both engines in
                                # parallel and store via the idle sync ring
                                nc.scalar.activation(
                                    dst[:, 0:512], pe[:, 0:512],
                                    mybir.ActivationFunctionType.Relu)
                                nc.vector.tensor_scalar_max(
                                    dst[:, 512:1024], pe[:, 512:1024], 0.0)
                                t0 = (tgc + ti) * 128
                                nc.sync.dma_start(out_ap[t0:t0 + 128, :],
                                                  dst)
                        tiles.append(tile_go)
                        ti += 1
                        tg += 1

                def store(last):
                    if not last:
                        dram = out_ap[tgc * 128: (tgc + tpc) * 128, :].rearrange(
                            "(i p) f -> p i f", p=128)
                        nc.gpsimd.dma_start(
                            dram, ob[:].rearrange("p (i f) -> p i f", f=WD))

                return tiles, store

            # software pipeline: chunk c's expansion tiles are interleaved
            # into chunk c+1's P chain (~3 P matmuls per exp tile), so the PE
            # stream never waits on PSUM drains.
            mms, drain = make_P(chunks[0], first=True)
            for go in mms:
                go()
            drain()
            for ci, c in enumerate(chunks):
                tiles, store = make_exp(c)
                if ci + 1 < len(chunks):
                    mms, drain = make_P(chunks[ci + 1])
                    lead = 4
                    for go in mms[:lead]:
                        go()
                    rem = mms[lead:]
                    nt = len(tiles)
                    done = 0
                    for j, tgo in enumerate(tiles):
                        upto = ((j + 1) * len(rem)) // (nt + 1)
                        while done < upto:
                            rem[done]()
                            done += 1
                        tgo()
                    while done < len(rem):
                        rem[done]()
                        done += 1
                    drain()
                else:
                    for tgo in tiles:
                        tgo(last=True)
                store(ci == len(chunks) - 1)
    nc.compile()
    return nc


def kernel(word_emb, char_ids, word_ids, E, W_ih, b_ih, b_hh, W_lin, b_lin,
           _timing=None, _trace_cores=None):
    word_emb = np.asarray(word_emb, np.float32)
    char_ids = np.asarray(char_ids, np.int32)
    word_ids = np.asarray(word_ids, np.int32)
    E = np.asarray(E, np.float32)
    W_ih = np.asarray(W_ih, np.float32)
    b_ih = np.asarray(b_ih, np.float32)
    b_hh = np.asarray(b_hh, np.float32)
    W_lin = np.asarray(W_lin, np.float32)
    b_lin = np.asarray(b_lin, np.float32)

    T = char_ids.shape[0]
    NW = word_emb.shape[0]

    HB = _hb_table(E, W_ih, b_ih, b_hh, W_lin, b_lin)
    A = np.ascontiguousarray(W_lin[:, :WD])

    word_bin, word_slot, nbins, bin_chars, bin_words = _pack_bins(word_ids, NW)
    assert bin_words.max() <= SUB_WORD_CAP and bin_chars.max() <= SUB_CHAR_CAP

    # deal sub-bins to (core, slot) by descending char count; slot ->
    # (chunk, half).  Rank ordering keeps per-slot char counts uniform across
    # cores so the shared tiles-per-slot wastes little padding.
    rank_of_bin = np.empty(nbins, np.int64)
    rank_of_bin[np.argsort(-bin_chars, kind="stable")] = np.arange(nbins)
    core_of_bin = (rank_of_bin % NCORES).astype(np.int32)
    slot_of_bin = (rank_of_bin // NCORES).astype(np.int32)
    nslots = nbins // NCORES
    assert nslots % 2 == 0
    nch = nslots // 2

    # chars sorted by (bin, word slot) -> contiguous per bin, word-major
    cb = word_bin[word_ids]
    cslot = word_slot[word_ids]
    ckey = cb.astype(np.int64) * 512 + cslot
    corder = np.argsort(ckey, kind="stable")
    per_bin = np.bincount(cb, minlength=nbins)
    bstart = np.concatenate([[0], np.cumsum(per_bin)])

    slot_cnt = np.zeros((NCORES, nslots), np.int64)
    for b in range(nbins):
        slot_cnt[core_of_bin[b], slot_of_bin[b]] = per_bin[b]
    tiles_per_slot = np.ceil(slot_cnt.max(axis=0) / 128).astype(np.int64)
    # drop empty trailing chunks
    while nch > 1 and tiles_per_slot[2 * nch - 2: 2 * nch].sum() == 0:
        nch -= 1
    nslots = 2 * nch
    tiles_per_slot = tiles_per_slot[:nslots]
    tiles_L = tiles_per_slot[0::2]
    tiles_R = tiles_per_slot[1::2]
    ntiles = int(tiles_per_slot.sum())
    slot_tile_base = np.concatenate([[0], np.cumsum(tiles_per_slot)])
    # chunk tile base for the device program piece layout
    chunk_tiles = tiles_L + tiles_R
    chunk_tile_base = np.concatenate([[0], np.cumsum(chunk_tiles)])

    AT3 = np.ascontiguousarray(A.T.reshape(8, 128, WD))
    at_host = np.empty((128, 8 * WD), NP_DT)
    for n in range(2):
        for k in range(8):
            at_host[:, n * 4 * WD + k * 512: n * 4 * WD + (k + 1) * 512] = \
                AT3[k][:, n * 512:(n + 1) * 512].astype(NP_DT)
    HBq = HB.astype(NP_DT)

    bin_of_cs = {}  # (core, slot) -> bin id
    for b in range(nbins):
        bin_of_cs[(int(core_of_bin[b]), int(slot_of_bin[b]))] = b

    in_maps = []
    origs = []
    for m in range(NCORES):
        weTb = np.zeros((nch, 128, WD), NP_DT)
        G = np.zeros((128, ntiles * 128), NP_DT)
        orig = np.full(ntiles * 128, -1, np.int64)
        for s in range(nslots):
            b = bin_of_cs.get((m, s))
            if b is None:
                continue
            c, half = s // 2, s % 2
            lo, hi = bstart[b], bstart[b + 1]
            chars = corder[lo:hi]
            wlist = np.nonzero(word_bin == b)[0]
            wlist = wlist[np.argsort(word_slot[wlist])]
            nwb = len(wlist)
            if nwb:
                rows = word_emb[wlist]  # [nwb, WD]
                blk = rows.T.reshape(8, 128, nwb).transpose(1, 0, 2)
                weTb[c].reshape(128, 8, 128)[
                    :, :, 64 * half: 64 * half + nwb] = blk.astype(NP_DT)
            q = np.arange(len(chars))
            col = slot_tile_base[s] * 128 + q
            ws = cslot[chars]
            cid = char_ids[chars]
            if half == 0:   # L sub-bin: sel rows 0-63, HB rows 64-103
                G[ws, col] = 1.0
                G[64 + cid, col] = 1.0
            else:           # R sub-bin: sel rows 64-127, HB rows 0-39
                G[64 + ws, col] = 1.0
                G[cid, col] = 1.0
            orig[col] = chars
        in_maps.append({
            "weTb": weTb,
            "atb": at_host,
            "gmat": G,
            "hbt": HBq,
        })
        origs.append(orig)

    nc = _build_program(nch, tiles_L, tiles_R, chunk_tile_base)
    kwargs = {}
    if _trace_cores is not None:
        kwargs = dict(trace=True, trace_cores=_trace_cores)
    res = run_bass_kernel_spmd(nc, in_maps, core_ids=list(range(NCORES)),
                               **kwargs)
    if _timing is not None:
        _timing["exec_time_ns"] = res.exec_time_ns
        _timing["results"] = res

    out = np.empty((T, WD), np.float32)
    for m in range(NCORES):
        o = np.asarray(res.results[m]["out"]).astype(np.float32)
        v = origs[m] >= 0
        out[origs[m][v]] = o[v]
    return out
